# revision 1
# baseline (speedup 1.0000x reference)
"""Trainium2 Bass kernel for nn_Condensation: 10 sequential masked-Gaussian-blur
composites over a [16,3,768,768] image, data-parallel over 8 NeuronCores.

Strategy (per core, 2 images = 6 image-channels):
  - out state kept resident in SBUF as bf16 [128, 6, 768] per image-channel;
    img is converted f32->bf16 on host and DMA'd straight into the state.
  - Per drop, work restricted to the mask's support box (mask < ~1e-5 outside).
  - Separable blur done as two banded-matmul passes on TensorE (bf16, f32 PSUM):
      pass A: vT[w, h'] = sum_h om[h, w] * M^T[h, h']   (image block stationary)
      pass B: B[h', w'] = sum_w vT[w, h'] * M^T[w, w']  (vT block stationary)
    streaming only the nonzero band of M^T per stationary block (PSUM start=True
    zeroes the whole 2KB bank, so unequal accumulation windows are fine).
  - Masks and conv matrices are computed exactly on host (numpy) from the
    runtime positions/radius inputs and DMA'd in per drop.
  - Composite out += m * (B - out): sub/mul on VectorE (bf16 2x), final add on
    GpSimd; PSUM evictions mostly on ScalarE with a fraction on VectorE.
"""
import numpy as np
import ml_dtypes

NUM_DROPS = 10
MIN_R, MAX_R = 60.0, 80.0
BETA = 1.8
BLUR_RADII = [11.3535, 17.9381, 5.7966, 10.8586, 5.5301, 15.9075, 12.3225, 13.4871, 6.6639, 9.5413]


def _ksize(r):
    k = int(2 * r) + 1
    return k + 1 if k % 2 == 0 else k


KSIZES = [_ksize(r) for r in BLUR_RADII]
H = W = 768
B_TOTAL, C = 16, 3
N_CORES = 8
B_LOC = B_TOTAL // N_CORES          # 2 images per core
IC = B_LOC * C                      # 6 image-channels per core
P = 128
NBLK = H // P                       # 6 h-blocks per image
EPS = 1e-5                          # mask support threshold

_bf16 = ml_dtypes.bfloat16


def _conv_matrix(sigma, ksize, n=768):
    """n x n matrix Kmat with blur_1d(x) = Kmat @ x, matching the reference
    (correlation with normalized gaussian, 'reflect' padding)."""
    half = (ksize - 1) * 0.5
    xs = np.linspace(-half, half, ksize)
    pdf = np.exp(-0.5 * (xs / np.float64(sigma)) ** 2)
    k1 = (pdf / pdf.sum()).astype(np.float32).astype(np.float64)
    pad = ksize // 2
    Kmat = np.zeros((n, n), dtype=np.float64)
    idx = np.arange(n)[:, None] + np.arange(ksize)[None, :] - pad   # [n, ksize]
    idx = np.abs(idx)
    idx = np.where(idx >= n, 2 * n - 2 - idx, idx)
    np.add.at(Kmat, (np.repeat(np.arange(n), ksize), idx.ravel()),
              np.tile(k1, n))
    return Kmat.astype(np.float32)


class _Drop:
    pass


def _drop_meta(positions, radius):
    """Host-side per-drop geometry + tensors (shared across cores)."""
    pos = np.clip(np.asarray(positions, np.float32), -1.0, 1.0)
    rad = np.clip(np.asarray(radius, np.float32), MIN_R, MAX_R)
    hv = np.arange(H, dtype=np.float32)[:, None]
    wv = np.arange(W, dtype=np.float32)[None, :]
    drops = []
    for j in range(NUM_DROPS):
        x0 = (pos[j, 0] + 1.0) / 2.0 * W
        y0 = (pos[j, 1] + 1.0) / 2.0 * H
        wr = rad[j]
        hr = wr * np.float32(0.8)
        ks = KSIZES[j]
        p = ks // 2
        s = float(np.sqrt((-np.log(EPS)) ** (1.0 / BETA)))
        h0 = max(0, int(np.floor(y0 - s * hr)))
        h1 = min(H, int(np.ceil(y0 + s * hr)) + 1)
        w0 = max(0, int(np.floor(x0 - s * wr)))
        w1 = min(W, int(np.ceil(x0 + s * wr)) + 1)
        w0 &= ~1
        w1 = min(W, (w1 + 1) & ~1)
        HB0, HB1 = h0 // P, (h1 + P - 1) // P
        d = _Drop()
        d.j, d.p = j, p
        d.h0, d.h1, d.w0, d.w1 = h0, h1, w0, w1
        d.HB0, d.HBn = HB0, HB1 - HB0
        d.HBs, d.HBw = HB0 * P, (HB1 - HB0) * P
        d.Wr = w1 - w0
        # tight (even, not necessarily 128-aligned) w-chunking covering
        # [w0-p, w1+p) for the blur input / vT partitions
        lo = max(0, w0 - p) & ~1
        hi = min(W, w1 + p)
        d.WBn = (hi - lo + P - 1) // P
        d.wL = min(lo, W - d.WBn * P)
        d.Ww = d.WBn * P

        # mask over [HB rows] x [wL:wL+Ww], zero outside support
        dd = (hv[d.HBs:d.HBs + d.HBw] - y0) ** 2 / hr ** 2 + \
             (wv[:, d.wL:d.wL + d.Ww] - x0) ** 2 / wr ** 2
        m = np.clip(np.exp(-(dd.astype(np.float32) ** np.float32(BETA)) + np.float32(1e-10)), 0.0, 1.0)
        mz = np.zeros_like(m)
        mz[h0 - d.HBs:h1 - d.HBs, w0 - d.wL:w1 - d.wL] = \
            m[h0 - d.HBs:h1 - d.HBs, w0 - d.wL:w1 - d.wL]
        # SBUF layout [part, 2, hb, w] (duplicated along the ic-pair dim)
        m1 = np.ascontiguousarray(
            mz.reshape(d.HBn, P, d.Ww).transpose(1, 0, 2)).astype(_bf16)
        d.m_np = np.ascontiguousarray(
            np.broadcast_to(m1[:, None], (P, 2, d.HBn, d.Ww)))

        MT = _conv_matrix(BLUR_RADII[j], ks).T    # MT[src, dst]
        kv = MT[d.HBs:d.HBs + d.HBw, d.HBs:d.HBs + d.HBw]      # [h, h']
        d.kv_np = np.ascontiguousarray(
            kv.reshape(d.HBn, P, d.HBw).transpose(1, 0, 2)).astype(_bf16)
        kh = MT[d.wL:d.wL + d.Ww, w0:w1]                       # [w, w']
        d.kh_np = np.ascontiguousarray(
            kh.reshape(d.WBn, P, d.Wr).transpose(1, 0, 2)).astype(_bf16)
        drops.append(d)
    return drops


def _build_program(drops):
    from contextlib import ExitStack
    from concourse import bacc, tile, mybir

    f32 = mybir.dt.float32
    bf16 = mybir.dt.bfloat16

    nc = bacc.Bacc("TRN2", target_bir_lowering=False, debug=False,
                   num_devices=N_CORES)
    img_d = nc.declare_dram_parameter("img", [B_LOC, C, H, W], bf16, False)
    out_d = nc.declare_dram_parameter("out", [B_LOC, C, H, W], bf16, True)
    dparams = []
    for d in drops:
        m_d = nc.declare_dram_parameter(f"m{d.j}", [P, 2, d.HBn, d.Ww], bf16, False)
        kv_d = nc.declare_dram_parameter(f"kv{d.j}", [P, d.HBn, d.HBw], bf16, False)
        kh_d = nc.declare_dram_parameter(f"kh{d.j}", [P, d.WBn, d.Wr], bf16, False)
        dparams.append((m_d, kv_d, kh_d))

    NG = IC // 2   # 3 groups of 2 image-channels

    evict_ctr = [0]

    def evict(dst, src):
        # PSUM->SBUF eviction: mostly ScalarE (closer to PSUM), every 6th on
        # VectorE to balance engine load
        evict_ctr[0] += 1
        if evict_ctr[0] % 6 == 0:
            nc.vector.tensor_copy(dst, src)
        else:
            nc.scalar.copy(dst, src)

    with tile.TileContext(nc) as tc, ExitStack() as ctx:
        outp = ctx.enter_context(tc.tile_pool(name="out_state", bufs=1))
        out_s = [outp.tile([P, 2, NBLK, W], bf16, name=f"state{g}", tag=f"state{g}")
                 for g in range(NG)]
        dp = ctx.enter_context(tc.tile_pool(name="dropin", bufs=3))
        wp = ctx.enter_context(tc.tile_pool(name="work", bufs=3))
        ppa = ctx.enter_context(tc.tile_pool(name="psa", bufs=2, space="PSUM"))
        ppb = ctx.enter_context(tc.tile_pool(name="psb", bufs=2, space="PSUM"))

        # img viewed as [p, (b c), n, w]; one DMA per ic-pair
        img_r = img_d.ap().rearrange("b c (n p) w -> p (b c) n w", p=P)
        out_r = out_d.ap().rearrange("b c (n p) w -> p (b c) n w", p=P)
        for g in range(NG):
            nc.sync.dma_start(out=out_s[g][:], in_=img_r[:, 2 * g:2 * g + 2])

        # ---- drops (param DMAs on the scalar queue so they don't serialize
        #      behind the image loads on the sync queue)
        for d, (m_d, kv_d, kh_d) in zip(drops, dparams):
            m_t = dp.tile([P, 2, d.HBn, d.Ww], bf16, tag="m")
            kv_t = dp.tile([P, d.HBn, d.HBw], bf16, tag="kv")
            kh_t = dp.tile([P, d.WBn, d.Wr], bf16, tag="kh")
            nc.scalar.dma_start(out=m_t[:], in_=m_d.ap()[:])
            nc.scalar.dma_start(out=kv_t[:], in_=kv_d.ap()[:])
            nc.scalar.dma_start(out=kh_t[:], in_=kh_d.ap()[:])
            for g in range(NG):
                om = wp.tile([P, 2, d.HBn, d.Ww], bf16, tag="om")
                om_eng = nc.vector if (d.j + g) % 2 == 0 else nc.gpsimd
                om_eng.tensor_mul(
                    om[:], m_t[:],
                    out_s[g][:, :, d.HB0:d.HB0 + d.HBn, d.wL:d.wL + d.Ww])
                # pass A: vT[w-chunk, h'] over HB window, banded
                vts = []
                for wc in range(d.WBn):
                    psa = ppa.tile([P, 2, 512], f32, tag="psa")
                    for j in range(2):
                        for k in range(d.HBn):
                            a = max(0, P * k - d.p)
                            b_ = min(d.HBw, P * (k + 1) + d.p)
                            nc.tensor.matmul(
                                psa[:, j, a:b_],
                                lhsT=om[:, j, k, wc * P:(wc + 1) * P],
                                rhs=kv_t[:, k, a:b_],
                                start=(k == 0), stop=(k == d.HBn - 1))
                    vt = wp.tile([P, 2, d.HBw], bf16, tag="vt", bufs=8)
                    evict(vt[:], psa[:, :, 0:d.HBw])
                    vts.append(vt)
                # pass B + composite pipelined per h'-block
                mr = m_t[:, :, :, d.w0 - d.wL:d.w0 - d.wL + d.Wr]
                for hb in range(d.HBn):
                    psb = ppb.tile([P, 2, 512], f32, tag="psb")
                    for j in range(2):
                        for wc in range(d.WBn):
                            wabs = d.wL + P * wc
                            a = max(0, wabs - d.p - d.w0)
                            b_ = min(d.Wr, wabs + P + d.p - d.w0)
                            nc.tensor.matmul(
                                psb[:, j, a:b_],
                                lhsT=vts[wc][:, j, hb * P:(hb + 1) * P],
                                rhs=kh_t[:, wc, a:b_],
                                start=(wc == 0), stop=(wc == d.WBn - 1))
                    Bsh = wp.tile([P, 2, d.Wr], bf16, tag="Bs", bufs=6)
                    evict(Bsh[:], psb[:, :, 0:d.Wr])
                    # composite: out += m * (B - out) on rows of this block
                    osl = out_s[g][:, :, d.HB0 + hb, d.w0:d.w1]
                    t1 = wp.tile([P, 2, d.Wr], bf16, tag="t1", bufs=6)
                    nc.vector.tensor_sub(t1[:], Bsh[:], osl)
                    t2 = wp.tile([P, 2, d.Wr], bf16, tag="t2", bufs=6)
                    nc.vector.tensor_mul(t2[:], mr[:, :, hb, :], t1[:])
                    add_eng = nc.vector if (d.j + g + hb) % 2 == 0 else nc.gpsimd
                    add_eng.tensor_add(osl, osl, t2[:])

        # ---- store state (bf16) -> out (bf16), one DMA per ic-pair
        for g in range(NG):
            nc.sync.dma_start(out=out_r[:, 2 * g:2 * g + 2], in_=out_s[g][:])

    nc.compile()
    return nc


_CACHE = {}


def _get_program(positions, radius):
    key = (np.asarray(positions, np.float32).tobytes(),
           np.asarray(radius, np.float32).tobytes())
    if key not in _CACHE:
        drops = _drop_meta(positions, radius)
        _CACHE[key] = (_build_program(drops), drops)
    return _CACHE[key]


def kernel(img, positions, radius, _want_trace=False, **_kw):
    from concourse.bass_utils import run_bass_kernel_spmd
    img = np.asarray(img, np.float32)
    assert img.shape == (B_TOTAL, C, H, W)
    nc, drops = _get_program(positions, radius)

    shards = np.ascontiguousarray(img.astype(_bf16)).reshape(
        N_CORES, B_LOC, C, H, W)
    base = {}
    for d in drops:
        base[f"m{d.j}"] = d.m_np
        base[f"kv{d.j}"] = d.kv_np
        base[f"kh{d.j}"] = d.kh_np
    in_maps = [dict(base, img=shards[i]) for i in range(N_CORES)]
    res = run_bass_kernel_spmd(nc, in_maps, core_ids=list(range(N_CORES)),
                               trace=_want_trace)
    out = np.concatenate([res.results[i]["out"] for i in range(N_CORES)], axis=0)
    out = out.reshape(B_TOTAL, C, H, W).astype(np.float32)
    if _want_trace:
        return out, res
    return out

